# revision 11
# baseline (speedup 1.0000x reference)
"""Trainium2 Bass kernel for nn_DynamicMatrix (gnn_message_passing).

Math (per reference):
  Q = x @ W_Q; K = x @ W_K                      # [B,E,V,KS]
  s = (Q @ K^T) / sqrt(KS) + eye(V)             # [B,E,V,V]
  a = softmax(s, axis=E); t = softmax(theta, axis=E)
  out = relu(a - t)

Key transforms used here:
  - eye(V) is constant along the softmax axis (E) -> softmax-invariant -> dropped.
  - 1/sqrt(KS) = 1/8 folded into W_Q (exact power-of-two scale).
  - theta is constant along E (fill=ones) -> t == 1/E exactly -> scalar bias.
  - softmax uses an approximate per-(v,w) max m~ (computed from a cheap
    fp16 QhKh-only score pass); any constant shift cancels exactly in
    softmax, m~ only needs to be within ~±80 of the true max.
  - x is pre-transposed on host to [B,E,P2,V] so the contraction dim (P2)
    lands on SBUF partitions with 800B-contiguous DMA descriptors.

Sharding: data-parallel over B across 8 cores (2 batches/core); W replicated.
"""

import os
import numpy as np

STAGE = int(os.environ.get("K_STAGE", "9"))
B, E, V, P2, KS = 16, 64, 200, 256, 64
NCORES = 8
B_LOC = B // NCORES
VCHUNKS = [(0, 128), (128, 72)]  # (v offset, v size)

_NC = None


def _build_nc():
    import concourse.bacc as bacc
    import concourse.tile as tile
    from concourse import mybir

    F32 = mybir.dt.float32
    F16 = mybir.dt.float16
    AL = mybir.AluOpType

    nc = bacc.Bacc("TRN2", target_bir_lowering=False, debug=False,
                   num_devices=NCORES)
    xt = nc.dram_tensor("xt", [B_LOC, E, P2, V], F32, kind="ExternalInput")
    wqk = nc.dram_tensor("wqk", [P2, 128], F32, kind="ExternalInput")
    out = nc.dram_tensor("out", [B_LOC, E, V, V], F32, kind="ExternalOutput")
    # relu threshold (softmax(theta) value, normally 1/64), passed as a
    # [128,1] per-partition scalar so non-constant-theta fallback stays on host
    cth = nc.dram_tensor("cth", [128, 1], F32, kind="ExternalInput")

    with tile.TileContext(nc) as tc:
        with (
            tc.tile_pool(name="xt_p", bufs=3) as xt_p,
            tc.tile_pool(name="w_p", bufs=1) as w_p,
            tc.tile_pool(name="qk_p", bufs=1) as qk_p,
            tc.tile_pool(name="su_p", bufs=2) as su_p,
            tc.tile_pool(name="tree_p", bufs=1) as tree_p,
            tc.tile_pool(name="mz_p", bufs=2) as mz_p,
            tc.tile_pool(name="a_p", bufs=2) as a_p,
            tc.tile_pool(name="o_p", bufs=2) as o_p,
            tc.tile_pool(name="ps", bufs=2, space="PSUM") as ps,
        ):
            w_sb = w_p.tile([128, 2, 128], F32, tag="w")
            nc.sync.dma_start(out=w_sb[:], in_=wqk.rearrange("(h p) m -> p h m", p=128))
            c_sb = w_p.tile([128, 1], F32, tag="c")
            nc.sync.dma_start(out=c_sb[:], in_=cth[:])

            for b in range(B_LOC):
                # ---- QK: psum_e = [Q'(e)^T ; K(e)^T] ; split to fp16 hi/lo --
                # SBUF layout: parity on partition halves: even e at parts 0-63,
                # odd e at parts 64-127; pair index = e//2 along free dim.
                qh = qk_p.tile([128, 32, V], F16, tag="qh")
                ql = qk_p.tile([128, 32, V], F16, tag="ql")
                kh = qk_p.tile([128, 32, V], F16, tag="kh")
                kl = qk_p.tile([128, 32, V], F16, tag="kl")
                for g in range(E // 8):  # 8 e's (= 4 pairs) per psum tile
                    xt_t = xt_p.tile([128, 8, 2, V], F32, tag="xt")
                    nc.sync.dma_start(
                        out=xt_t[:],
                        in_=xt[b, g * 8:(g + 1) * 8].rearrange(
                            "e (h p) v -> p e h v", p=128),
                    )
                    pq = ps.tile([128, 8, 256], F32, tag="ps")
                    for s in range(8):
                        for h in range(2):
                            nc.tensor.matmul(
                                pq[:, s, 0:V],
                                w_sb[:, h, :],
                                xt_t[:, s, h, :],
                                start=(h == 0), stop=(h == 1),
                            )
                    p0 = g * 4  # first pair slot in SBUF
                    # 4 streams x (hi on ACT, lo on DVE STT)
                    for (dst, psl, dsl) in (
                        ((qh, ql), (0, 64), (0, 64)),    # Q even: direct
                        ((qh, ql), (0, 64), (64, 128)),  # Q odd: shift +64
                        ((kh, kl), (64, 128), (0, 64)),  # K even: shift -64
                        ((kh, kl), (64, 128), (64, 128)),  # K odd: direct
                    ):
                        par = 0 if dsl[0] == 0 else 1
                        src = pq[psl[0]:psl[1], par:8:2, 0:V]
                        hi = dst[0][dsl[0]:dsl[1], p0:p0 + 4, :]
                        lo = dst[1][dsl[0]:dsl[1], p0:p0 + 4, :]
                        nc.scalar.copy(out=hi, in_=src)
                        nc.vector.scalar_tensor_tensor(
                            out=lo, in0=src, scalar=1.0, in1=hi,
                            op0=AL.mult, op1=AL.subtract,
                        )

                if STAGE < 2:
                    ot = o_p.tile([128, 1], F32, tag="dbg")
                    nc.vector.tensor_copy(ot[:], c_sb[:])
                    nc.sync.dma_start(out=out[b, 0, 0:128, 0:1], in_=ot[:])
                    continue
                for ci, (voff, vsz) in enumerate(VCHUNKS):
                    # ---- round 1: hh-only scores -> s~ fp16 ----
                    st = su_p.tile([vsz, E, V], F16, tag="su")
                    for g in range(E // 16):
                        g16 = g * 16
                        for par in range(2):
                            p1 = ps.tile([128, 8, 256], F32, tag="ps")
                            r0, r1_ = 64 * par, 64 * par + 64
                            for s in range(8):
                                e = g16 + par + 2 * s
                                pr = e // 2
                                nc.tensor.matmul(
                                    p1[0:vsz, s, 0:V],
                                    qh[r0:r1_, pr, voff:voff + vsz],
                                    kh[r0:r1_, pr, :],
                                    start=True, stop=True,
                                )
                            nc.scalar.copy(
                                out=st[:, g16 + par:g16 + 16:2, :],
                                in_=p1[0:vsz, 0:8, 0:V],
                            )

                    # ---- tree max over E -> m~ [vsz, 1, V] f32 ----
                    m = mz_p.tile([vsz, 1, V], F32, tag="m")
                    if os.environ.get("K_NOTREE"):
                        nc.vector.tensor_copy(m[:], st[:, 0:1, :])
                    else:
                        tm = tree_p.tile([vsz, 32, V], F16, tag="tree")
                        nc.vector.tensor_max(tm[:], st[:, 0:32, :], st[:, 32:64, :])
                        for wdt in (16, 8, 4, 2):
                            nc.vector.tensor_max(
                                tm[:, 0:wdt, :], tm[:, 0:wdt, :], tm[:, wdt:2 * wdt, :])
                        nc.vector.tensor_max(m[:], tm[:, 0:1, :], tm[:, 1:2, :])

                    if STAGE < 3:
                        ot = o_p.tile([vsz, 1, V], F32, tag="dbgm")
                        nc.vector.tensor_copy(ot[:], m[:])
                        nc.sync.dma_start(out=out[b, 0, voff:voff + vsz, :], in_=ot[:, 0, :])
                        continue
                    # ---- round 2: full scores; u = s - m~ (fp16) ----
                    u = su_p.tile([vsz, E, V], F16, tag="su")
                    for g in range(E // 16):
                        g16 = g * 16
                        for par in range(2):
                            p2t = ps.tile([128, 8, 256], F32, tag="ps")
                            r0, r1_ = 64 * par, 64 * par + 64
                            for s in range(8):
                                e = g16 + par + 2 * s
                                pr = e // 2
                                qhs = qh[r0:r1_, pr, voff:voff + vsz]
                                qls = ql[r0:r1_, pr, voff:voff + vsz]
                                o = p2t[0:vsz, s, 0:V]
                                nc.tensor.matmul(o, qhs, kh[r0:r1_, pr, :],
                                                 start=True, stop=False)
                                nc.tensor.matmul(o, qhs, kl[r0:r1_, pr, :],
                                                 start=False, stop=False)
                                nc.tensor.matmul(o, qls, kh[r0:r1_, pr, :],
                                                 start=False, stop=True)
                            nc.vector.scalar_tensor_tensor(
                                out=u[:, g16 + par:g16 + 16:2, :],
                                in0=p2t[0:vsz, 0:8, 0:V],
                                scalar=1.0,
                                in1=m[:].to_broadcast((vsz, 8, V)),
                                op0=AL.mult, op1=AL.subtract,
                            )

                    if STAGE < 4:
                        ot = o_p.tile([vsz, 1, V], F32, tag="dbgm")
                        nc.vector.tensor_copy(ot[:], u[:, 0:1, :])
                        nc.sync.dma_start(out=out[b, 0, voff:voff + vsz, :], in_=ot[:, 0, :])
                        continue
                    # ---- exp (fp16) ----
                    Ex = su_p.tile([vsz, E, V], F16, tag="su")
                    nc.scalar.activation(
                        out=Ex[:], in_=u[:],
                        func=mybir.ActivationFunctionType.Exp,
                    )

                    # ---- tree sum -> Z; zr = 1/Z ----
                    ts_ = tree_p.tile([vsz, 32, V], F16, tag="tree")
                    nc.vector.tensor_add(ts_[:], Ex[:, 0:32, :], Ex[:, 32:64, :])
                    nc.vector.tensor_add(
                        ts_[:, 0:16, :], ts_[:, 0:16, :], ts_[:, 16:32, :])
                    zf = mz_p.tile([vsz, 8, V], F32, tag="zf")
                    nc.vector.tensor_add(zf[:], ts_[:, 0:8, :], ts_[:, 8:16, :])
                    nc.vector.tensor_add(zf[:, 0:4, :], zf[:, 0:4, :], zf[:, 4:8, :])
                    nc.vector.tensor_add(zf[:, 0:2, :], zf[:, 0:2, :], zf[:, 2:4, :])
                    z = mz_p.tile([vsz, 1, V], F32, tag="z")
                    nc.vector.tensor_add(z[:], zf[:, 0:1, :], zf[:, 1:2, :])
                    zr = mz_p.tile([vsz, 1, V], F32, tag="zr")
                    nc.vector.reciprocal(out=zr[:], in_=z[:])
                    zrh = mz_p.tile([vsz, 1, V], F16, tag="zrh")
                    nc.vector.tensor_copy(zrh[:], zr[:])

                    # ---- a = E * zr ; out = relu(a - c) ; store ----
                    for sl in range(8):  # 8-e slabs
                        es = sl * 8
                        at = a_p.tile([vsz, 8, V], F16, tag="a")
                        nc.vector.tensor_mul(
                            at[:], Ex[:, es:es + 8, :],
                            zrh[:].to_broadcast((vsz, 8, V)),
                        )
                        ot = o_p.tile([vsz, 8, V], F32, tag="o")
                        nc.vector.tensor_scalar(
                            out=ot[:], in0=at[:],
                            scalar1=c_sb[0:vsz, :], scalar2=0.0,
                            op0=AL.subtract, op1=AL.max,
                        )
                        nc.sync.dma_start(
                            out=out[b, es:es + 8, voff:voff + vsz, :].rearrange(
                                "e v w -> v e w"),
                            in_=ot[:],
                        )
    nc.compile()
    return nc


def _get_nc():
    global _NC
    if _NC is None:
        _NC = _build_nc()
    return _NC


def kernel(x, W_Q, W_K, theta):
    from concourse.bass_utils import run_bass_kernel_spmd

    x = np.asarray(x, dtype=np.float32)
    W_Q = np.asarray(W_Q, dtype=np.float32)
    W_K = np.asarray(W_K, dtype=np.float32)
    theta = np.asarray(theta, dtype=np.float32)

    # t = softmax(theta, axis=1); theta is constant along axis 1 by spec,
    # so t is a constant plane. Verify and fall back to host combine if not.
    th = theta.astype(np.float64)
    th -= th.max(axis=1, keepdims=True)
    t_full = np.exp(th)
    t_full /= t_full.sum(axis=1, keepdims=True)
    t_const = float(t_full.flat[0])
    const_theta = bool(np.all(np.abs(t_full - t_const) < 1e-12))
    c_val = t_const if const_theta else 0.0

    wqk = np.concatenate([W_Q / 8.0, W_K], axis=1).astype(np.float32)
    cth = np.full((128, 1), c_val, dtype=np.float32)

    nc = _get_nc()
    in_maps = []
    for c in range(NCORES):
        xs = x[c * B_LOC:(c + 1) * B_LOC]
        xt = np.ascontiguousarray(xs.transpose(0, 1, 3, 2))
        in_maps.append({"xt": xt, "wqk": wqk, "cth": cth})

    res = run_bass_kernel_spmd(nc, in_maps, core_ids=list(range(NCORES)))
    outs = [res.results[c]["out"] for c in range(NCORES)]
    y = np.concatenate(outs, axis=0)

    if not const_theta:
        # device computed softmax a (c=0 -> relu(a) == a since a >= 0)
        y = np.maximum(y - t_full.astype(np.float32), 0.0)
    return y


# revision 12
# speedup vs baseline: 1.1525x; 1.1525x over previous
"""Trainium2 Bass kernel for nn_DynamicMatrix (gnn_message_passing).

Math (per reference):
  Q = x @ W_Q; K = x @ W_K                      # [B,E,V,KS]
  s = (Q @ K^T) / sqrt(KS) + eye(V)             # [B,E,V,V]
  a = softmax(s, axis=E); t = softmax(theta, axis=E)
  out = relu(a - t)

Key transforms used here:
  - eye(V) is constant along the softmax axis (E) -> softmax-invariant -> dropped.
  - 1/sqrt(KS) = 1/8 folded into W_Q (exact power-of-two scale).
  - theta is constant along E (fill=ones) -> t == 1/E exactly -> scalar bias.
  - softmax uses an approximate per-(v,w) max m~ (computed from a cheap
    fp16 QhKh-only score pass); any constant shift cancels exactly in
    softmax, m~ only needs to be within ~±80 of the true max.
  - x is pre-transposed on host to [B,E,P2,V] so the contraction dim (P2)
    lands on SBUF partitions with 800B-contiguous DMA descriptors.

Sharding: data-parallel over B across 8 cores (2 batches/core); W replicated.
"""

import os
import numpy as np

STAGE = int(os.environ.get("K_STAGE", "9"))
B, E, V, P2, KS = 16, 64, 200, 256, 64
NCORES = 8
B_LOC = B // NCORES
VCHUNKS = [(0, 128), (128, 72)]  # (v offset, v size)

_NC = None


def _build_nc():
    import concourse.bacc as bacc
    import concourse.tile as tile
    from concourse import mybir

    F32 = mybir.dt.float32
    F16 = mybir.dt.float16
    AL = mybir.AluOpType

    nc = bacc.Bacc("TRN2", target_bir_lowering=False, debug=False,
                   num_devices=NCORES)
    xt = nc.dram_tensor("xt", [B_LOC, E, P2, V], F32, kind="ExternalInput")
    wqk = nc.dram_tensor("wqk", [P2, 128], F32, kind="ExternalInput")
    out = nc.dram_tensor("out", [B_LOC, E, V, V], F32, kind="ExternalOutput")
    # relu threshold (softmax(theta) value, normally 1/64), passed as a
    # [128,1] per-partition scalar so non-constant-theta fallback stays on host
    cth = nc.dram_tensor("cth", [128, 1], F32, kind="ExternalInput")

    with tile.TileContext(nc) as tc:
        with (
            tc.tile_pool(name="xt_p", bufs=3) as xt_p,
            tc.tile_pool(name="w_p", bufs=1) as w_p,
            tc.tile_pool(name="qk_p", bufs=1) as qk_p,
            tc.tile_pool(name="su_p", bufs=2) as su_p,
            tc.tile_pool(name="tree_p", bufs=1) as tree_p,
            tc.tile_pool(name="mz_p", bufs=2) as mz_p,
            tc.tile_pool(name="a_p", bufs=2) as a_p,
            tc.tile_pool(name="o_p", bufs=2) as o_p,
            tc.tile_pool(name="ps", bufs=2, space="PSUM") as ps,
        ):
            w_sb = w_p.tile([128, 2, 128], F32, tag="w")
            nc.sync.dma_start(out=w_sb[:], in_=wqk.rearrange("(h p) m -> p h m", p=128))
            c_sb = w_p.tile([128, 1], F32, tag="c")
            nc.sync.dma_start(out=c_sb[:], in_=cth[:])

            for b in range(B_LOC):
                # ---- QK: psum_e = [Q'(e)^T ; K(e)^T] ; split to fp16 hi/lo --
                # SBUF layout: parity on partition halves: even e at parts 0-63,
                # odd e at parts 64-127; pair index = e//2 along free dim.
                qh = qk_p.tile([128, 32, V], F16, tag="qh")
                ql = qk_p.tile([128, 32, V], F16, tag="ql")
                kh = qk_p.tile([128, 32, V], F16, tag="kh")
                kl = qk_p.tile([128, 32, V], F16, tag="kl")
                for g in range(E // 8):  # 8 e's (= 4 pairs) per psum tile
                    xt_t = xt_p.tile([128, 8, 2, V], F32, tag="xt")
                    nc.sync.dma_start(
                        out=xt_t[:],
                        in_=xt[b, g * 8:(g + 1) * 8].rearrange(
                            "e (h p) v -> p e h v", p=128),
                    )
                    pq = ps.tile([128, 8, 256], F32, tag="ps")
                    for s in range(8):
                        for h in range(2):
                            nc.tensor.matmul(
                                pq[:, s, 0:V],
                                w_sb[:, h, :],
                                xt_t[:, s, h, :],
                                start=(h == 0), stop=(h == 1),
                            )
                    p0 = g * 4  # first pair slot in SBUF
                    # 4 streams x (hi on ACT, lo on DVE STT)
                    for (dst, psl, dsl) in (
                        ((qh, ql), (0, 64), (0, 64)),    # Q even: direct
                        ((qh, ql), (0, 64), (64, 128)),  # Q odd: shift +64
                        ((kh, kl), (64, 128), (0, 64)),  # K even: shift -64
                        ((kh, kl), (64, 128), (64, 128)),  # K odd: direct
                    ):
                        par = 0 if dsl[0] == 0 else 1
                        src = pq[psl[0]:psl[1], par:8:2, 0:V]
                        hi = dst[0][dsl[0]:dsl[1], p0:p0 + 4, :]
                        lo = dst[1][dsl[0]:dsl[1], p0:p0 + 4, :]
                        nc.scalar.copy(out=hi, in_=src)
                        nc.vector.scalar_tensor_tensor(
                            out=lo, in0=src, scalar=1.0, in1=hi,
                            op0=AL.mult, op1=AL.subtract,
                        )

                if STAGE < 2:
                    ot = o_p.tile([128, 1], F32, tag="dbg")
                    nc.vector.tensor_copy(ot[:], c_sb[:])
                    nc.sync.dma_start(out=out[b, 0, 0:128, 0:1], in_=ot[:])
                    continue
                for ci, (voff, vsz) in enumerate(VCHUNKS):
                    # ---- round 1: hh-only scores -> s~ fp16 ----
                    st = su_p.tile([vsz, E, V], F16, tag="su")
                    gm = mz_p.tile([vsz, 4, V], F16, tag="gm")
                    t8 = tree_p.tile([vsz, 8, V], F16, tag="t8")
                    for g in range(E // 16):
                        g16 = g * 16
                        for par in range(2):
                            p1 = ps.tile([128, 8, 256], F32, tag="ps")
                            r0, r1_ = 64 * par, 64 * par + 64
                            for s in range(8):
                                e = g16 + par + 2 * s
                                pr = e // 2
                                nc.tensor.matmul(
                                    p1[0:vsz, s, 0:V],
                                    qh[r0:r1_, pr, voff:voff + vsz],
                                    kh[r0:r1_, pr, :],
                                    start=True, stop=True,
                                )
                            nc.scalar.copy(
                                out=st[:, g16 + par:g16 + 16:2, :],
                                in_=p1[0:vsz, 0:8, 0:V],
                            )
                        nc.vector.tensor_max(
                            t8[:], st[:, g16:g16 + 8, :], st[:, g16 + 8:g16 + 16, :])
                        for wdt in (4, 2):
                            nc.vector.tensor_max(
                                t8[:, 0:wdt, :], t8[:, 0:wdt, :], t8[:, wdt:2 * wdt, :])
                        nc.vector.tensor_max(
                            gm[:, g:g + 1, :], t8[:, 0:1, :], t8[:, 1:2, :])

                    # ---- tree max over E -> m~ [vsz, 1, V] f32 ----
                    m = mz_p.tile([vsz, 1, V], F32, tag="m")
                    nc.vector.tensor_max(gm[:, 0:2, :], gm[:, 0:2, :], gm[:, 2:4, :])
                    nc.vector.tensor_max(m[:], gm[:, 0:1, :], gm[:, 1:2, :])

                    if STAGE < 3:
                        ot = o_p.tile([vsz, 1, V], F32, tag="dbgm")
                        nc.vector.tensor_copy(ot[:], m[:])
                        nc.sync.dma_start(out=out[b, 0, voff:voff + vsz, :], in_=ot[:, 0, :])
                        continue
                    # ---- round 2: full scores; u = s - m~ (fp16) ----
                    u = su_p.tile([vsz, E, V], F16, tag="su")
                    for g in range(E // 16):
                        g16 = g * 16
                        for par in range(2):
                            p2t = ps.tile([128, 8, 256], F32, tag="ps")
                            r0, r1_ = 64 * par, 64 * par + 64
                            for s in range(8):
                                e = g16 + par + 2 * s
                                pr = e // 2
                                qhs = qh[r0:r1_, pr, voff:voff + vsz]
                                qls = ql[r0:r1_, pr, voff:voff + vsz]
                                o = p2t[0:vsz, s, 0:V]
                                nc.tensor.matmul(o, qhs, kh[r0:r1_, pr, :],
                                                 start=True, stop=False)
                                nc.tensor.matmul(o, qhs, kl[r0:r1_, pr, :],
                                                 start=False, stop=False)
                                nc.tensor.matmul(o, qls, kh[r0:r1_, pr, :],
                                                 start=False, stop=True)
                            nc.vector.scalar_tensor_tensor(
                                out=u[:, g16 + par:g16 + 16:2, :],
                                in0=p2t[0:vsz, 0:8, 0:V],
                                scalar=1.0,
                                in1=m[:].to_broadcast((vsz, 8, V)),
                                op0=AL.mult, op1=AL.subtract,
                            )

                    if STAGE < 4:
                        ot = o_p.tile([vsz, 1, V], F32, tag="dbgm")
                        nc.vector.tensor_copy(ot[:], u[:, 0:1, :])
                        nc.sync.dma_start(out=out[b, 0, voff:voff + vsz, :], in_=ot[:, 0, :])
                        continue
                    # ---- exp (fp16) per 16-e group, with group sums ----
                    Ex = su_p.tile([vsz, E, V], F16, tag="su")
                    gz = mz_p.tile([vsz, 4, V], F32, tag="gz")
                    s8 = tree_p.tile([vsz, 8, V], F16, tag="t8")
                    for g in range(E // 16):
                        g16 = g * 16
                        nc.scalar.activation(
                            out=Ex[:, g16:g16 + 16, :], in_=u[:, g16:g16 + 16, :],
                            func=mybir.ActivationFunctionType.Exp,
                        )
                        nc.vector.tensor_add(
                            s8[:], Ex[:, g16:g16 + 8, :], Ex[:, g16 + 8:g16 + 16, :])
                        for wdt in (4, 2):
                            nc.vector.tensor_add(
                                s8[:, 0:wdt, :], s8[:, 0:wdt, :], s8[:, wdt:2 * wdt, :])
                        nc.vector.tensor_add(
                            gz[:, g:g + 1, :], s8[:, 0:1, :], s8[:, 1:2, :])
                    z = mz_p.tile([vsz, 1, V], F32, tag="z")
                    nc.vector.tensor_add(gz[:, 0:2, :], gz[:, 0:2, :], gz[:, 2:4, :])
                    nc.vector.tensor_add(z[:], gz[:, 0:1, :], gz[:, 1:2, :])
                    zr = mz_p.tile([vsz, 1, V], F32, tag="zr")
                    nc.vector.reciprocal(out=zr[:], in_=z[:])
                    zrh = mz_p.tile([vsz, 1, V], F16, tag="zrh")
                    nc.vector.tensor_copy(zrh[:], zr[:])

                    # ---- a = E * zr ; out = relu(a - c) ; store ----
                    for sl in range(8):  # 8-e slabs
                        es = sl * 8
                        at = a_p.tile([vsz, 8, V], F16, tag="a")
                        nc.vector.tensor_mul(
                            at[:], Ex[:, es:es + 8, :],
                            zrh[:].to_broadcast((vsz, 8, V)),
                        )
                        ot = o_p.tile([vsz, 8, V], F32, tag="o")
                        nc.vector.tensor_scalar(
                            out=ot[:], in0=at[:],
                            scalar1=c_sb[0:vsz, :], scalar2=0.0,
                            op0=AL.subtract, op1=AL.max,
                        )
                        nc.sync.dma_start(
                            out=out[b, es:es + 8, voff:voff + vsz, :].rearrange(
                                "e v w -> v e w"),
                            in_=ot[:],
                        )
    nc.compile()
    return nc


def _get_nc():
    global _NC
    if _NC is None:
        _NC = _build_nc()
    return _NC


def kernel(x, W_Q, W_K, theta):
    from concourse.bass_utils import run_bass_kernel_spmd

    x = np.asarray(x, dtype=np.float32)
    W_Q = np.asarray(W_Q, dtype=np.float32)
    W_K = np.asarray(W_K, dtype=np.float32)
    theta = np.asarray(theta, dtype=np.float32)

    # t = softmax(theta, axis=1); theta is constant along axis 1 by spec,
    # so t is a constant plane. Verify and fall back to host combine if not.
    th = theta.astype(np.float64)
    th -= th.max(axis=1, keepdims=True)
    t_full = np.exp(th)
    t_full /= t_full.sum(axis=1, keepdims=True)
    t_const = float(t_full.flat[0])
    const_theta = bool(np.all(np.abs(t_full - t_const) < 1e-12))
    c_val = t_const if const_theta else 0.0

    wqk = np.concatenate([W_Q / 8.0, W_K], axis=1).astype(np.float32)
    cth = np.full((128, 1), c_val, dtype=np.float32)

    nc = _get_nc()
    in_maps = []
    for c in range(NCORES):
        xs = x[c * B_LOC:(c + 1) * B_LOC]
        xt = np.ascontiguousarray(xs.transpose(0, 1, 3, 2))
        in_maps.append({"xt": xt, "wqk": wqk, "cth": cth})

    res = run_bass_kernel_spmd(nc, in_maps, core_ids=list(range(NCORES)))
    outs = [res.results[c]["out"] for c in range(NCORES)]
    y = np.concatenate(outs, axis=0)

    if not const_theta:
        # device computed softmax a (c=0 -> relu(a) == a since a >= 0)
        y = np.maximum(y - t_full.astype(np.float32), 0.0)
    return y


# revision 14
# speedup vs baseline: 1.1678x; 1.0132x over previous
"""Trainium2 Bass kernel for nn_DynamicMatrix (gnn_message_passing).

Math (per reference):
  Q = x @ W_Q; K = x @ W_K                      # [B,E,V,KS]
  s = (Q @ K^T) / sqrt(KS) + eye(V)             # [B,E,V,V]
  a = softmax(s, axis=E); t = softmax(theta, axis=E)
  out = relu(a - t)

Key transforms used here:
  - eye(V) is constant along the softmax axis (E) -> softmax-invariant -> dropped.
  - 1/sqrt(KS) = 1/8 folded into W_Q (exact power-of-two scale).
  - theta is constant along E (fill=ones) -> t == 1/E exactly -> scalar bias.
  - softmax uses an approximate per-(v,w) max m~ (computed from a cheap
    fp16 QhKh-only score pass); any constant shift cancels exactly in
    softmax, m~ only needs to be within ~±80 of the true max.
  - x is pre-transposed on host to [B,E,P2,V] so the contraction dim (P2)
    lands on SBUF partitions with 800B-contiguous DMA descriptors.

Sharding: data-parallel over B across 8 cores (2 batches/core); W replicated.
"""

import numpy as np

B, E, V, P2, KS = 16, 64, 200, 256, 64
NCORES = 8
B_LOC = B // NCORES
VCHUNKS = [(0, 128), (128, 72)]  # (v offset, v size)

_NC = None


def _build_nc():
    import concourse.bacc as bacc
    import concourse.tile as tile
    from concourse import mybir

    F32 = mybir.dt.float32
    F16 = mybir.dt.float16
    AL = mybir.AluOpType

    nc = bacc.Bacc("TRN2", target_bir_lowering=False, debug=False,
                   num_devices=NCORES)
    xt = nc.dram_tensor("xt", [B_LOC, E, P2, V], F32, kind="ExternalInput")
    wqk = nc.dram_tensor("wqk", [P2, 128], F32, kind="ExternalInput")
    out = nc.dram_tensor("out", [B_LOC, E, V, V], F32, kind="ExternalOutput")
    # relu threshold (softmax(theta) value, normally 1/64), passed as a
    # [128,1] per-partition scalar so non-constant-theta fallback stays on host
    cth = nc.dram_tensor("cth", [128, 1], F32, kind="ExternalInput")

    with tile.TileContext(nc) as tc:
        with (
            tc.tile_pool(name="xt_p", bufs=2) as xt_p,
            tc.tile_pool(name="w_p", bufs=1) as w_p,
            tc.tile_pool(name="qk_p", bufs=1) as qk_p,
            tc.tile_pool(name="su_p", bufs=2) as su_p,
            tc.tile_pool(name="tree_p", bufs=1) as tree_p,
            tc.tile_pool(name="mz_p", bufs=3) as mz_p,
            tc.tile_pool(name="a_p", bufs=3) as a_p,
            tc.tile_pool(name="o_p", bufs=3) as o_p,
            tc.tile_pool(name="ps", bufs=2, space="PSUM") as ps,
        ):
            w_sb = w_p.tile([128, 2, 128], F32, tag="w")
            nc.sync.dma_start(out=w_sb[:], in_=wqk.rearrange("(h p) m -> p h m", p=128))
            c_sb = w_p.tile([128, 1], F32, tag="c")
            nc.sync.dma_start(out=c_sb[:], in_=cth[:])

            for b in range(B_LOC):
                # ---- QK: psum_e = [Q'(e)^T ; K(e)^T] ; split to fp16 hi/lo --
                # SBUF layout: parity on partition halves: even e at parts 0-63,
                # odd e at parts 64-127; pair index = e//2 along free dim.
                qh = qk_p.tile([128, 32, V], F16, tag="qh")
                ql = qk_p.tile([128, 32, V], F16, tag="ql")
                kh = qk_p.tile([128, 32, V], F16, tag="kh")
                kl = qk_p.tile([128, 32, V], F16, tag="kl")
                for g in range(E // 8):  # 8 e's (= 4 pairs) per psum tile
                    xt_t = xt_p.tile([128, 8, 2, V], F32, tag="xt")
                    nc.sync.dma_start(
                        out=xt_t[:],
                        in_=xt[b, g * 8:(g + 1) * 8].rearrange(
                            "e (h p) v -> p e h v", p=128),
                    )
                    pq = ps.tile([128, 8, 256], F32, tag="ps")
                    for s in range(8):
                        for h in range(2):
                            nc.tensor.matmul(
                                pq[:, s, 0:V],
                                w_sb[:, h, :],
                                xt_t[:, s, h, :],
                                start=(h == 0), stop=(h == 1),
                            )
                    p0 = g * 4  # first pair slot in SBUF
                    # 4 streams x (hi on ACT, lo on DVE STT)
                    for (dst, psl, dsl) in (
                        ((qh, ql), (0, 64), (0, 64)),    # Q even: direct
                        ((qh, ql), (0, 64), (64, 128)),  # Q odd: shift +64
                        ((kh, kl), (64, 128), (0, 64)),  # K even: shift -64
                        ((kh, kl), (64, 128), (64, 128)),  # K odd: direct
                    ):
                        par = 0 if dsl[0] == 0 else 1
                        src = pq[psl[0]:psl[1], par:8:2, 0:V]
                        hi = dst[0][dsl[0]:dsl[1], p0:p0 + 4, :]
                        lo = dst[1][dsl[0]:dsl[1], p0:p0 + 4, :]
                        nc.scalar.copy(out=hi, in_=src)
                        nc.vector.scalar_tensor_tensor(
                            out=lo, in0=src, scalar=1.0, in1=hi,
                            op0=AL.mult, op1=AL.subtract,
                        )

                for ci, (voff, vsz) in enumerate(VCHUNKS):
                    # ---- round 1: hh-only scores -> s~ fp16 ----
                    st = su_p.tile([vsz, E, V], F16, tag="su")
                    gm = mz_p.tile([vsz, 4, V], F16, tag="gm")
                    t8 = tree_p.tile([vsz, 8, V], F16, tag="t8")
                    for g in range(E // 16):
                        g16 = g * 16
                        for par in range(2):
                            p1 = ps.tile([128, 8, 256], F32, tag="ps")
                            r0, r1_ = 64 * par, 64 * par + 64
                            for s in range(8):
                                e = g16 + par + 2 * s
                                pr = e // 2
                                nc.tensor.matmul(
                                    p1[0:vsz, s, 0:V],
                                    qh[r0:r1_, pr, voff:voff + vsz],
                                    kh[r0:r1_, pr, :],
                                    start=True, stop=True,
                                )
                            nc.scalar.copy(
                                out=st[:, g16 + par:g16 + 16:2, :],
                                in_=p1[0:vsz, 0:8, 0:V],
                            )
                        nc.vector.tensor_max(
                            t8[:], st[:, g16:g16 + 8, :], st[:, g16 + 8:g16 + 16, :])
                        for wdt in (4, 2):
                            nc.vector.tensor_max(
                                t8[:, 0:wdt, :], t8[:, 0:wdt, :], t8[:, wdt:2 * wdt, :])
                        nc.vector.tensor_max(
                            gm[:, g:g + 1, :], t8[:, 0:1, :], t8[:, 1:2, :])

                    # ---- tree max over E -> m~ [vsz, 1, V] f32 ----
                    m = mz_p.tile([vsz, 1, V], F32, tag="m")
                    nc.vector.tensor_max(gm[:, 0:2, :], gm[:, 0:2, :], gm[:, 2:4, :])
                    nc.vector.tensor_max(m[:], gm[:, 0:1, :], gm[:, 1:2, :])

                    # ---- round 2: full scores; u = s - m~ (fp16) ----
                    u = su_p.tile([vsz, E, V], F16, tag="su")
                    for g in range(E // 16):
                        g16 = g * 16
                        for par in range(2):
                            p2t = ps.tile([128, 8, 256], F32, tag="ps")
                            r0, r1_ = 64 * par, 64 * par + 64
                            for s in range(8):
                                e = g16 + par + 2 * s
                                pr = e // 2
                                qhs = qh[r0:r1_, pr, voff:voff + vsz]
                                qls = ql[r0:r1_, pr, voff:voff + vsz]
                                o = p2t[0:vsz, s, 0:V]
                                nc.tensor.matmul(o, qhs, kh[r0:r1_, pr, :],
                                                 start=True, stop=False)
                                nc.tensor.matmul(o, qhs, kl[r0:r1_, pr, :],
                                                 start=False, stop=False)
                                nc.tensor.matmul(o, qls, kh[r0:r1_, pr, :],
                                                 start=False, stop=True)
                            nc.vector.scalar_tensor_tensor(
                                out=u[:, g16 + par:g16 + 16:2, :],
                                in0=p2t[0:vsz, 0:8, 0:V],
                                scalar=1.0,
                                in1=m[:].to_broadcast((vsz, 8, V)),
                                op0=AL.mult, op1=AL.subtract,
                            )

                    # ---- exp (fp16) per 16-e group, with group sums ----
                    Ex = su_p.tile([vsz, E, V], F16, tag="su")
                    gz = mz_p.tile([vsz, 4, V], F32, tag="gz")
                    s8 = tree_p.tile([vsz, 8, V], F16, tag="t8")
                    for g in range(E // 16):
                        g16 = g * 16
                        nc.scalar.activation(
                            out=Ex[:, g16:g16 + 16, :], in_=u[:, g16:g16 + 16, :],
                            func=mybir.ActivationFunctionType.Exp,
                        )
                        nc.vector.tensor_add(
                            s8[:], Ex[:, g16:g16 + 8, :], Ex[:, g16 + 8:g16 + 16, :])
                        for wdt in (4, 2):
                            nc.vector.tensor_add(
                                s8[:, 0:wdt, :], s8[:, 0:wdt, :], s8[:, wdt:2 * wdt, :])
                        nc.vector.tensor_add(
                            gz[:, g:g + 1, :], s8[:, 0:1, :], s8[:, 1:2, :])
                    z = mz_p.tile([vsz, 1, V], F32, tag="z")
                    nc.vector.tensor_add(gz[:, 0:2, :], gz[:, 0:2, :], gz[:, 2:4, :])
                    nc.vector.tensor_add(z[:], gz[:, 0:1, :], gz[:, 1:2, :])
                    zr = mz_p.tile([vsz, 1, V], F32, tag="zr")
                    nc.vector.reciprocal(out=zr[:], in_=z[:])
                    zrh = mz_p.tile([vsz, 1, V], F16, tag="zrh")
                    nc.vector.tensor_copy(zrh[:], zr[:])

                    # ---- a = E * zr ; out = relu(a - c) ; store ----
                    for sl in range(8):  # 8-e slabs
                        es = sl * 8
                        at = a_p.tile([vsz, 8, V], F16, tag="a")
                        nc.vector.tensor_mul(
                            at[:], Ex[:, es:es + 8, :],
                            zrh[:].to_broadcast((vsz, 8, V)),
                        )
                        ot = o_p.tile([vsz, 8, V], F32, tag="o")
                        nc.vector.tensor_scalar(
                            out=ot[:], in0=at[:],
                            scalar1=c_sb[0:vsz, :], scalar2=0.0,
                            op0=AL.subtract, op1=AL.max,
                        )
                        nc.sync.dma_start(
                            out=out[b, es:es + 8, voff:voff + vsz, :].rearrange(
                                "e v w -> v e w"),
                            in_=ot[:],
                        )
    nc.compile()
    return nc


def _get_nc():
    global _NC
    if _NC is None:
        _NC = _build_nc()
    return _NC


def kernel(x, W_Q, W_K, theta):
    from concourse.bass_utils import run_bass_kernel_spmd

    x = np.asarray(x, dtype=np.float32)
    W_Q = np.asarray(W_Q, dtype=np.float32)
    W_K = np.asarray(W_K, dtype=np.float32)
    theta = np.asarray(theta, dtype=np.float32)

    # t = softmax(theta, axis=1); theta is constant along axis 1 by spec,
    # so t is a constant plane. Verify and fall back to host combine if not.
    th = theta.astype(np.float64)
    th -= th.max(axis=1, keepdims=True)
    t_full = np.exp(th)
    t_full /= t_full.sum(axis=1, keepdims=True)
    t_const = float(t_full.flat[0])
    const_theta = bool(np.all(np.abs(t_full - t_const) < 1e-12))
    c_val = t_const if const_theta else 0.0

    wqk = np.concatenate([W_Q / 8.0, W_K], axis=1).astype(np.float32)
    cth = np.full((128, 1), c_val, dtype=np.float32)

    nc = _get_nc()
    in_maps = []
    for c in range(NCORES):
        xs = x[c * B_LOC:(c + 1) * B_LOC]
        xt = np.ascontiguousarray(xs.transpose(0, 1, 3, 2))
        in_maps.append({"xt": xt, "wqk": wqk, "cth": cth})

    res = run_bass_kernel_spmd(nc, in_maps, core_ids=list(range(NCORES)))
    outs = [res.results[c]["out"] for c in range(NCORES)]
    y = np.concatenate(outs, axis=0)

    if not const_theta:
        # device computed softmax a (c=0 -> relu(a) == a since a >= 0)
        y = np.maximum(y - t_full.astype(np.float32), 0.0)
    return y


# revision 17
# speedup vs baseline: 1.1749x; 1.0060x over previous
"""Trainium2 Bass kernel for nn_DynamicMatrix (gnn_message_passing).

Math (per reference):
  Q = x @ W_Q; K = x @ W_K                      # [B,E,V,KS]
  s = (Q @ K^T) / sqrt(KS) + eye(V)             # [B,E,V,V]
  a = softmax(s, axis=E); t = softmax(theta, axis=E)
  out = relu(a - t)

Key transforms used here:
  - eye(V) is constant along the softmax axis (E) -> softmax-invariant -> dropped.
  - 1/sqrt(KS) = 1/8 folded into W_Q (exact power-of-two scale).
  - theta is constant along E (fill=ones) -> t == 1/E exactly -> scalar bias.
  - softmax uses an approximate per-(v,w) max m~ (computed from a cheap
    fp16 QhKh-only score pass); any constant shift cancels exactly in
    softmax, m~ only needs to be within ~±80 of the true max.
  - x is pre-transposed on host to [B,E,P2,V] so the contraction dim (P2)
    lands on SBUF partitions with 800B-contiguous DMA descriptors.

Sharding: data-parallel over B across 8 cores (2 batches/core); W replicated.
"""

import numpy as np

B, E, V, P2, KS = 16, 64, 200, 256, 64
NCORES = 8
B_LOC = B // NCORES
VCHUNKS = [(0, 128), (128, 72)]  # (v offset, v size)

_NC = None


def _build_nc():
    import concourse.bacc as bacc
    import concourse.tile as tile
    from concourse import mybir

    F32 = mybir.dt.float32
    F16 = mybir.dt.float16
    AL = mybir.AluOpType

    nc = bacc.Bacc("TRN2", target_bir_lowering=False, debug=False,
                   num_devices=NCORES)
    xt = nc.dram_tensor("xt", [B_LOC, E, P2, V], F32, kind="ExternalInput")
    wqk = nc.dram_tensor("wqk", [P2, 128], F32, kind="ExternalInput")
    out = nc.dram_tensor("out", [B_LOC, E, V, V], F32, kind="ExternalOutput")
    # relu threshold (softmax(theta) value, normally 1/64), passed as a
    # [128,1] per-partition scalar so non-constant-theta fallback stays on host
    cth = nc.dram_tensor("cth", [128, 1], F32, kind="ExternalInput")

    with tile.TileContext(nc) as tc:
        with (
            tc.tile_pool(name="xt_p", bufs=2) as xt_p,
            tc.tile_pool(name="w_p", bufs=1) as w_p,
            tc.tile_pool(name="qk_p", bufs=1) as qk_p,
            tc.tile_pool(name="su_p", bufs=2) as su_p,
            tc.tile_pool(name="tree_p", bufs=1) as tree_p,
            tc.tile_pool(name="mz_p", bufs=2) as mz_p,
            tc.tile_pool(name="stg_p", bufs=3) as stg_p,
            tc.tile_pool(name="a_p", bufs=3) as a_p,
            tc.tile_pool(name="o_p", bufs=3) as o_p,
            tc.tile_pool(name="ps", bufs=2, space="PSUM") as ps,
        ):
            w_sb = w_p.tile([128, 2, 128], F32, tag="w")
            nc.sync.dma_start(out=w_sb[:], in_=wqk.rearrange("(h p) m -> p h m", p=128))
            c_sb = w_p.tile([128, 1], F32, tag="c")
            nc.sync.dma_start(out=c_sb[:], in_=cth[:])

            for b in range(B_LOC):
                # ---- QK: psum_e = [Q'(e)^T ; K(e)^T] ; split to fp16 hi/lo --
                # SBUF layout: parity on partition halves: even e at parts 0-63,
                # odd e at parts 64-127; pair index = e//2 along free dim.
                qh = qk_p.tile([128, 32, V], F16, tag="qh")
                ql = qk_p.tile([128, 32, V], F16, tag="ql")
                kh = qk_p.tile([128, 32, V], F16, tag="kh")
                kl = qk_p.tile([128, 32, V], F16, tag="kl")
                for g in range(E // 8):  # 8 e's (= 4 pairs) per psum tile
                    xt_t = xt_p.tile([128, 8, 2, V], F32, tag="xt")
                    nc.sync.dma_start(
                        out=xt_t[:],
                        in_=xt[b, g * 8:(g + 1) * 8].rearrange(
                            "e (h p) v -> p e h v", p=128),
                    )
                    pq = ps.tile([128, 8, 256], F32, tag="ps")
                    for s in range(8):
                        for h in range(2):
                            nc.tensor.matmul(
                                pq[:, s, 0:V],
                                w_sb[:, h, :],
                                xt_t[:, s, h, :],
                                start=(h == 0), stop=(h == 1),
                            )
                    p0 = g * 4  # first pair slot in SBUF
                    # 4 streams x (hi on ACT, lo on DVE STT)
                    for (dst, psl, dsl) in (
                        ((qh, ql), (0, 64), (0, 64)),    # Q even: direct
                        ((qh, ql), (0, 64), (64, 128)),  # Q odd: shift +64
                        ((kh, kl), (64, 128), (0, 64)),  # K even: shift -64
                        ((kh, kl), (64, 128), (64, 128)),  # K odd: direct
                    ):
                        par = 0 if dsl[0] == 0 else 1
                        src = pq[psl[0]:psl[1], par:8:2, 0:V]
                        hi = dst[0][dsl[0]:dsl[1], p0:p0 + 4, :]
                        lo = dst[1][dsl[0]:dsl[1], p0:p0 + 4, :]
                        nc.scalar.copy(out=hi, in_=src)
                        nc.vector.scalar_tensor_tensor(
                            out=lo, in0=src, scalar=1.0, in1=hi,
                            op0=AL.mult, op1=AL.subtract,
                        )

                for ci, (voff, vsz) in enumerate(VCHUNKS):
                    # ---- round 1: hh-only scores -> s~ fp16 ----
                    gm = mz_p.tile([vsz, 4, V], F16, tag="gm")
                    for g in range(E // 16):
                        g16 = g * 16
                        stg = stg_p.tile([vsz, 16, V], F16, tag="stg")
                        for par in range(2):
                            p1 = ps.tile([128, 8, 256], F32, tag="ps")
                            r0, r1_ = 64 * par, 64 * par + 64
                            for s in range(8):
                                e = g16 + par + 2 * s
                                pr = e // 2
                                nc.tensor.matmul(
                                    p1[0:vsz, s, 0:V],
                                    qh[r0:r1_, pr, voff:voff + vsz],
                                    kh[r0:r1_, pr, :],
                                    start=True, stop=True,
                                )
                            nc.scalar.copy(
                                out=stg[:, par:16:2, :],
                                in_=p1[0:vsz, 0:8, 0:V],
                            )
                        t8 = stg_p.tile([vsz, 8, V], F16, tag="t8")
                        nc.vector.tensor_max(
                            t8[:], stg[:, 0:8, :], stg[:, 8:16, :])
                        for wdt in (4, 2):
                            nc.vector.tensor_max(
                                t8[:, 0:wdt, :], t8[:, 0:wdt, :], t8[:, wdt:2 * wdt, :])
                        nc.vector.tensor_max(
                            gm[:, g:g + 1, :], t8[:, 0:1, :], t8[:, 1:2, :])

                    # ---- tree max over E -> m~ [vsz, 1, V] f32 ----
                    m = mz_p.tile([vsz, 1, V], F32, tag="m")
                    nc.vector.tensor_max(gm[:, 0:2, :], gm[:, 0:2, :], gm[:, 2:4, :])
                    nc.vector.tensor_max(m[:], gm[:, 0:1, :], gm[:, 1:2, :])

                    # ---- round 2: full scores; u = s - m~ (fp16) ----
                    u = su_p.tile([vsz, E, V], F16, tag="su")
                    for g in range(E // 16):
                        g16 = g * 16
                        for par in range(2):
                            p2t = ps.tile([128, 8, 256], F32, tag="ps")
                            r0, r1_ = 64 * par, 64 * par + 64
                            for s in range(8):
                                e = g16 + par + 2 * s
                                pr = e // 2
                                qhs = qh[r0:r1_, pr, voff:voff + vsz]
                                qls = ql[r0:r1_, pr, voff:voff + vsz]
                                o = p2t[0:vsz, s, 0:V]
                                nc.tensor.matmul(o, qhs, kh[r0:r1_, pr, :],
                                                 start=True, stop=False)
                                nc.tensor.matmul(o, qhs, kl[r0:r1_, pr, :],
                                                 start=False, stop=False)
                                nc.tensor.matmul(o, qls, kh[r0:r1_, pr, :],
                                                 start=False, stop=True)
                            nc.vector.scalar_tensor_tensor(
                                out=u[:, g16 + par:g16 + 16:2, :],
                                in0=p2t[0:vsz, 0:8, 0:V],
                                scalar=1.0,
                                in1=m[:].to_broadcast((vsz, 8, V)),
                                op0=AL.mult, op1=AL.subtract,
                            )

                    # ---- exp (fp16) per 16-e group, with group sums ----
                    Ex = su_p.tile([vsz, E, V], F16, tag="su")
                    gz = mz_p.tile([vsz, 4, V], F32, tag="gz")
                    s8 = tree_p.tile([vsz, 8, V], F16, tag="t8")
                    for g in range(E // 16):
                        g16 = g * 16
                        nc.scalar.activation(
                            out=Ex[:, g16:g16 + 16, :], in_=u[:, g16:g16 + 16, :],
                            func=mybir.ActivationFunctionType.Exp,
                        )
                        nc.vector.tensor_add(
                            s8[:], Ex[:, g16:g16 + 8, :], Ex[:, g16 + 8:g16 + 16, :])
                        for wdt in (4, 2):
                            nc.vector.tensor_add(
                                s8[:, 0:wdt, :], s8[:, 0:wdt, :], s8[:, wdt:2 * wdt, :])
                        nc.vector.tensor_add(
                            gz[:, g:g + 1, :], s8[:, 0:1, :], s8[:, 1:2, :])
                    z = mz_p.tile([vsz, 1, V], F32, tag="z")
                    nc.vector.tensor_add(gz[:, 0:2, :], gz[:, 0:2, :], gz[:, 2:4, :])
                    nc.vector.tensor_add(z[:], gz[:, 0:1, :], gz[:, 1:2, :])
                    zr = mz_p.tile([vsz, 1, V], F32, tag="zr")
                    nc.vector.reciprocal(out=zr[:], in_=z[:])
                    zrh = mz_p.tile([vsz, 1, V], F16, tag="zrh")
                    nc.vector.tensor_copy(zrh[:], zr[:])

                    # ---- a = E * zr ; out = relu(a - c) ; store ----
                    for sl in range(8):  # 8-e slabs
                        es = sl * 8
                        at = a_p.tile([vsz, 8, V], F16, tag="a")
                        nc.vector.tensor_mul(
                            at[:], Ex[:, es:es + 8, :],
                            zrh[:].to_broadcast((vsz, 8, V)),
                        )
                        ot = o_p.tile([vsz, 8, V], F32, tag="o")
                        nc.vector.tensor_scalar(
                            out=ot[:], in0=at[:],
                            scalar1=c_sb[0:vsz, :], scalar2=0.0,
                            op0=AL.subtract, op1=AL.max,
                        )
                        nc.sync.dma_start(
                            out=out[b, es:es + 8, voff:voff + vsz, :].rearrange(
                                "e v w -> v e w"),
                            in_=ot[:],
                        )
    nc.compile()
    return nc


def _get_nc():
    global _NC
    if _NC is None:
        _NC = _build_nc()
    return _NC


def kernel(x, W_Q, W_K, theta):
    from concourse.bass_utils import run_bass_kernel_spmd

    x = np.asarray(x, dtype=np.float32)
    W_Q = np.asarray(W_Q, dtype=np.float32)
    W_K = np.asarray(W_K, dtype=np.float32)
    theta = np.asarray(theta, dtype=np.float32)

    # t = softmax(theta, axis=1); theta is constant along axis 1 by spec,
    # so t is a constant plane. Verify and fall back to host combine if not.
    th = theta.astype(np.float64)
    th -= th.max(axis=1, keepdims=True)
    t_full = np.exp(th)
    t_full /= t_full.sum(axis=1, keepdims=True)
    t_const = float(t_full.flat[0])
    const_theta = bool(np.all(np.abs(t_full - t_const) < 1e-12))
    c_val = t_const if const_theta else 0.0

    wqk = np.concatenate([W_Q / 8.0, W_K], axis=1).astype(np.float32)
    cth = np.full((128, 1), c_val, dtype=np.float32)

    nc = _get_nc()
    in_maps = []
    for c in range(NCORES):
        xs = x[c * B_LOC:(c + 1) * B_LOC]
        xt = np.ascontiguousarray(xs.transpose(0, 1, 3, 2))
        in_maps.append({"xt": xt, "wqk": wqk, "cth": cth})

    res = run_bass_kernel_spmd(nc, in_maps, core_ids=list(range(NCORES)))
    outs = [res.results[c]["out"] for c in range(NCORES)]
    y = np.concatenate(outs, axis=0)

    if not const_theta:
        # device computed softmax a (c=0 -> relu(a) == a since a >= 0)
        y = np.maximum(y - t_full.astype(np.float32), 0.0)
    return y
